# revision 1
# baseline (speedup 1.0000x reference)
"""Trainium2 Bass kernel for nn_Decoder: attention+LSTM decoder.

Math (reference):
  k = h_enc @ Wk.T + bk ; v = h_enc @ Wv.T + bv        [B, 8, 32]
  3 decoder steps: q = h @ Wq.T + bq
     score_t = q.k_t/sqrt(32) ; att = softmax_t
     ctx = sum_t att_t v_t ; (h, c) = LSTMCell(ctx, h, c)
  logits_s = h_s @ Wout.T + b_out ; out = log_softmax(logits)   [B, 3, 10]

Kernel algebra (host-side folds):
  score_t = h.(A x_t) + w.x_t  with A = Wq.T Wk/sqrt(H), w = Wk.T bq/sqrt(H)
  (bias terms u.h + bq.bk are t-independent -> dropped by softmax invariance)
  v-bias bv folded into gate bias: bg = b_ih + b_hh + W_ih @ bv
  sigmoid via tanh: sig(x) = (1+tanh(x/2))/2; factor-2 carries:
     Chat = 2c, Hhat = 2h; consumers of Hhat pre-scaled by 0.5.
  ssum-volley computes 32*sum_t(e_t); the 32 is folded into W_ih (x32).

Layout: feature-major t-packed tiles [128, n]: partition = 32*(t%4)+h,
two tiles (lo: t0-3, hi: t4-7), batch on the free dim. n = CHUNK.
All matmuls are 32x32 PE tiles via tile_position volleys.
LSTM state bands: ti@0 tf@1 to@2 (gate col order {i,f,o}; g separate psum
at band0), Chat carried at band1, tanh(c) and Hhat at band2.
"""

import numpy as np

import concourse.bass as bass
import concourse.bacc as bacc
import concourse.tile as tile
from concourse import mybir
from concourse.bass_utils import run_bass_kernel_spmd

H = 32
HT = 8
FT = 3
OD = 10
N_CORES = 8

BF = mybir.dt.bfloat16
F32 = mybir.dt.float32
AF = mybir.ActivationFunctionType
ALU = mybir.AluOpType

CHUNK = 512          # batch elements per inner chunk
GROUP = 8            # chunks per phaseA/phaseB super-group (bounds table swaps)

# wpack (bf16, [128, WCOLS]) column layout
ID128_OFF = 0        # 128 cols: identity 128 (input transposes)
AT_OFF = 128         # 32: A^T replicated per 32-band
KAP_OFF = 160        # 32: kappa weights w[i] replicated along M, per band
VT_OFF = 192         # 32: Wv^T replicated per band
ONES_OFF = 224       # 32: all-ones
I32_OFF = 256        # 32: eye(32) per band
HI32_OFF = 288       # 32: 0.5*eye(32) per band
WIH_OFF = 320        # 128: (32*W_ih).T replicated per band, col-blocks {i,f,o,g}
WHH_OFF = 448        # 128: (0.5*W_hh).T replicated per band, col-blocks {i,f,o,g}
WOUT_OFF = 576       # 32: (0.5*W_out).T padded to 32 out, replicated per band
CC_OFF = 608         # 32: [64,32] Chat' combine: rows0-31 I32, rows32-63 0.5*I32
WIHG_OFF = 640       # 128: [96,128] gates: rows0-31 (32*W_ih).T, 32-63 zero,
                     #      64-95 (0.5*W_hh).T; col-blocks {i,f,o,g}
WCOLS = 768

# fpack (f32, [128, FCOLS])
SV_OFF = 0           # tanh scale vec: rows 0-95 = 0.5 (i,f,o bands)
BT_OFF = 1           # tanh bias vec: rows 0-95 = 0.5*bg in {i,f,o} order
BG_OFF = 2           # rows 0-31: bg of the g-block (scale 1.0)
BOUT_OFF = 3         # b_out padded: rows 32s+o: b_out[o] if o<10 else -30
IDT_OFF = 4          # 96 cols: f32 identity[96] for output transposes
FCOLS = 4 + 96


def _pack_weights(Wq, bq, Wk, bk, Wv, bv, W_ih, b_ih, W_hh, b_hh, W_out, b_out):
    Wq, bq, Wk, bk, Wv, bv, W_ih, b_ih, W_hh, b_hh, W_out, b_out = [
        np.asarray(a, np.float32) for a in
        (Wq, bq, Wk, bk, Wv, bv, W_ih, b_ih, W_hh, b_hh, W_out, b_out)]
    s = 1.0 / np.sqrt(np.float32(H))
    A = (Wq.T @ Wk) * s                    # [32,32] score bilinear form
    w = (Wk.T @ bq) * s                    # [32]
    bg = b_ih + b_hh + W_ih @ bv           # [128] gate bias (i,f,g,o order)

    wp = np.zeros((128, WCOLS), np.float32)
    wp[:, ID128_OFF:ID128_OFF + 128] = np.eye(128)
    # source rows of W_ih/W_hh/bg: i=0:32, f=32:64, g=64:96, o=96:128
    gate_slices = (slice(0, 32), slice(32, 64), slice(96, 128), slice(64, 96))
    for r in range(4):
        P = slice(32 * r, 32 * r + 32)
        wp[P, AT_OFF:AT_OFF + 32] = A.T
        wp[P, KAP_OFF:KAP_OFF + 32] = np.tile(w[:, None], (1, 32))
        wp[P, VT_OFF:VT_OFF + 32] = Wv.T
        wp[P, ONES_OFF:ONES_OFF + 32] = 1.0
        wp[P, I32_OFF:I32_OFF + 32] = np.eye(32)
        wp[P, HI32_OFF:HI32_OFF + 32] = 0.5 * np.eye(32)
        for ci, gsl in enumerate(gate_slices):
            wp[P, WIH_OFF + 32 * ci:WIH_OFF + 32 * ci + 32] = (32.0 * W_ih[gsl]).T
            wp[P, WHH_OFF + 32 * ci:WHH_OFF + 32 * ci + 32] = (0.5 * W_hh[gsl]).T
        wp[P, WOUT_OFF:WOUT_OFF + OD] = (0.5 * W_out).T
    wp[0:32, CC_OFF:CC_OFF + 32] = np.eye(32)
    wp[32:64, CC_OFF:CC_OFF + 32] = 0.5 * np.eye(32)
    for ci, gsl in enumerate(gate_slices):
        wp[0:32, WIHG_OFF + 32 * ci:WIHG_OFF + 32 * ci + 32] = (32.0 * W_ih[gsl]).T
        wp[64:96, WIHG_OFF + 32 * ci:WIHG_OFF + 32 * ci + 32] = (0.5 * W_hh[gsl]).T

    fp = np.zeros((128, FCOLS), np.float32)
    fp[0:96, SV_OFF] = 0.5
    fp[0:96, BT_OFF] = 0.5 * np.concatenate([bg[0:32], bg[32:64], bg[96:128]])
    fp[0:32, BG_OFF] = bg[64:96]
    bout = np.zeros(128, np.float32)
    for s3 in range(FT):
        for o in range(32):
            bout[32 * s3 + o] = b_out[o] if o < OD else -30.0
    fp[:, BOUT_OFF] = bout
    fp[0:96, IDT_OFF:IDT_OFF + 96] = np.eye(96)
    return wp, fp


def build_program(Bshard: int) -> bass.Bass:
    assert Bshard % CHUNK == 0
    nchunks = Bshard // CHUNK
    nc = bacc.Bacc(trn_type="TRN2")
    x_d = nc.declare_dram_parameter("h_enc", [Bshard, HT, H], F32, isOutput=False)
    wp_d = nc.declare_dram_parameter("wpack", [128, WCOLS], BF, isOutput=False)
    fp_d = nc.declare_dram_parameter("fpack", [128, FCOLS], F32, isOutput=False)
    out_d = nc.declare_dram_parameter("out", [Bshard, FT, OD], F32, isOutput=True)
    with tile.TileContext(nc) as tc:
        _body(nc, tc, x_d, wp_d, fp_d, out_d, nchunks, CHUNK)
    nc.compile()
    return nc


def _split_matmul_waits(nc):
    """Walrus instruction structs fit one sync wait; move extras onto
    same-engine no-ops (each carrying a single wait) inserted just before."""
    for b in nc.m.functions[0].blocks:
        new = []
        for ins in b.instructions:
            si = ins.sync_info
            if (si is not None and len(si.on_wait) > 1
                    and not isinstance(ins, (mybir.InstEventSemaphore,
                                             mybir.InstNoOp))):
                for w in si.on_wait[:-1]:
                    nop = mybir.InstNoOp(
                        name=nc.get_next_instruction_name(), ins=[], outs=[],
                        engine=ins.engine,
                        sync_info=mybir.SyncInfo(on_wait=[w], on_update=[]))
                    nc.register_instruction(nop)
                    new.append(nop)
                ins.sync_info = mybir.SyncInfo(
                    on_wait=[si.on_wait[-1]], on_update=list(si.on_update))
            new.append(ins)
        b.instructions[:] = new


def _body(nc, tc, x_d, wp_d, fp_d, out_d, nchunks, n):
    from contextlib import ExitStack
    ctx = ExitStack()
    with ctx:
        singles = ctx.enter_context(tc.tile_pool(name="singles", bufs=1))
        sb_x = ctx.enter_context(tc.tile_pool(name="sb_x", bufs=2))
        sb_kv = ctx.enter_context(tc.tile_pool(name="sb_kv", bufs=2))
        sb_step = ctx.enter_context(tc.tile_pool(name="sb_step", bufs=2))
        sb_small = ctx.enter_context(tc.tile_pool(name="sb_small", bufs=3))
        sb_out = ctx.enter_context(tc.tile_pool(name="sb_out", bufs=2))
        ps_x = ctx.enter_context(tc.tile_pool(name="ps_x", bufs=1, space="PSUM"))
        ps_big = ctx.enter_context(tc.tile_pool(name="ps_big", bufs=3, space="PSUM"))
        ps_sm = ctx.enter_context(tc.tile_pool(name="ps_sm", bufs=4, space="PSUM"))

        wp = singles.tile([128, WCOLS], BF)
        nc.sync.dma_start(out=wp, in_=wp_d[:, :])
        fp = singles.tile([128, FCOLS], F32)
        nc.sync.dma_start(out=fp, in_=fp_d[:, :])

        ident = wp[:, ID128_OFF:ID128_OFF + 128]

        def volley_diag(out_ps, lhs_off, rhs, start, stop):
            # slots (r, r): per-band weight apply; out band r <- W @ rhs band r
            for r in range(4):
                P = slice(32 * r, 32 * r + 32)
                nc.tensor.matmul(
                    out_ps[P, :], wp[P, lhs_off:lhs_off + 32], rhs[P, :],
                    start=start, stop=stop, tile_position=(32 * r, 32 * r),
                    skip_group_check=True)

        ngroups = (nchunks + GROUP - 1) // GROUP
        for gi in range(ngroups):
            g0 = gi * GROUP
            gsz = min(GROUP, nchunks - g0)
            # Hhat stash for phase B; only band2 (partitions 64-95) is used.
            stash = sb_out.tile([96, GROUP * FT * n], BF, tag="stash")

            # ---------------- phase A ----------------
            for cj in range(gsz):
                ci = g0 + cj
                nsub = n // 128
                xb = sb_x.tile([128, nsub, 256], BF, tag="xb")
                xv = x_d[ci * n:(ci + 1) * n].rearrange(
                    "(i p) t h -> p i (t h)", p=128)
                nc.gpsimd.dma_start(out=xb, in_=xv)

                xt = []
                for half in range(2):
                    xp = ps_x.tile([128, n], BF, tag="px")
                    for i in range(nsub):
                        for t4 in range(4):
                            tglob = 4 * half + t4
                            nc.tensor.transpose(
                                xp[32 * t4:32 * t4 + 32, 128 * i:128 * i + 128],
                                xb[:, i, 32 * tglob:32 * tglob + 32],
                                ident,
                                tile_position=(0, 32 * t4),
                            )
                    xs = sb_x.tile([128, n], BF, tag=f"xt{half}")
                    if half == 0:
                        nc.vector.tensor_copy(xs, xp)
                    else:
                        nc.scalar.copy(out=xs, in_=xp)
                    xt.append(xs)

                ks, vs = [], []
                for half in range(2):
                    kp = ps_big.tile([128, n], F32, tag="pbig")
                    volley_diag(kp, AT_OFF, xt[half], True, True)
                    k_s = sb_kv.tile([128, n], BF, tag=f"ks{half}")
                    if half == 0:
                        nc.vector.tensor_copy(k_s, kp)
                    else:
                        nc.scalar.copy(out=k_s, in_=kp)
                    ks.append(k_s)
                for half in range(2):
                    vp = ps_big.tile([128, n], F32, tag="pbig")
                    volley_diag(vp, VT_OFF, xt[half], True, True)
                    v_s = sb_kv.tile([128, n], BF, tag=f"vs{half}")
                    if half == 0:
                        nc.vector.tensor_copy(v_s, vp)
                    else:
                        nc.scalar.copy(out=v_s, in_=vp)
                    vs.append(v_s)

                hprev = None       # Hhat slice [32, n] at band2 (partitions 64-95)
                slprev = None      # previous step's state slice [96, n]
                for s in range(1, FT + 1):
                    # ---- scores -> att weights (unnormalized exp) ----
                    if s > 1:
                        hr_p = ps_big.tile([128, n], F32, tag="pbig")
                        for c in range(4):
                            nc.tensor.matmul(
                                hr_p[32 * c:32 * c + 32, :],
                                wp[64:96, HI32_OFF:HI32_OFF + 32], hprev,
                                start=True, stop=True,
                                tile_position=(64, 32 * c),
                                skip_group_check=True)
                        hrep = sb_step.tile([128, n], BF, tag="hrep")
                        nc.scalar.copy(out=hrep, in_=hr_p)
                    es = []
                    for half in range(2):
                        scp = ps_big.tile([128, n], F32, tag="pbig")
                        volley_diag(scp, KAP_OFF, xt[half], True, s == 1)
                        if s > 1:
                            pt = sb_step.tile([128, n], BF, tag="pt")
                            nc.vector.tensor_mul(pt, ks[half], hrep)
                            volley_diag(scp, ONES_OFF, pt, False, True)
                        e_t = sb_step.tile([128, n], BF, tag=f"e{half}")
                        nc.scalar.activation(out=e_t, in_=scp, func=AF.Exp)
                        es.append(e_t)
                    # ---- context ----
                    cu_p = ps_sm.tile([32, n], F32, tag="psm")
                    for half in range(2):
                        q_t = sb_step.tile([128, n], BF, tag="qt")
                        nc.vector.tensor_mul(q_t, es[half], vs[half])
                        nc.tensor.matmul(
                            cu_p[:, :], wp[0:128, I32_OFF:I32_OFF + 32],
                            q_t[0:128, :], start=(half == 0), stop=(half == 1),
                            tile_position=(0, 0), skip_group_check=True)
                    ssum_p = ps_sm.tile([32, n], F32, tag="psm")
                    for half in range(2):
                        nc.tensor.matmul(
                            ssum_p[:, :], wp[0:128, ONES_OFF:ONES_OFF + 32],
                            es[half][0:128, :], start=(half == 0),
                            stop=(half == 1),
                            tile_position=(0, 0), skip_group_check=True)
                    rs = sb_small.tile([32, n], F32, tag="rs")
                    nc.vector.reciprocal(out=rs, in_=ssum_p)
                    # cx_s lands in the PREVIOUS step's state slice (band0) so
                    # the gates matmul can contract [cx; Chat; Hhat] in one go.
                    if s == 1:
                        cx = sb_small.tile([32, n], BF, tag="cx")
                    else:
                        cx = slprev[0:32, :]
                    nc.vector.tensor_tensor(out=cx, in0=cu_p, in1=rs,
                                            op=ALU.mult)

                    # ---- gates (psum bands {i,f,o}; g separate at band0) ----
                    gp = ps_sm.tile([96, n], F32, tag="psm")
                    gp_g = ps_sm.tile([32, n], F32, tag="psm")
                    if s == 1:
                        for c in range(3):
                            nc.tensor.matmul(
                                gp[32 * c:32 * c + 32, :],
                                wp[0:32, WIH_OFF + 32 * c:WIH_OFF + 32 * c + 32],
                                cx, start=True, stop=True,
                                tile_position=(0, 32 * c),
                                skip_group_check=True)
                        nc.tensor.matmul(
                            gp_g, wp[0:32, WIH_OFF + 96:WIH_OFF + 128],
                            cx, start=True, stop=True, tile_position=(0, 0),
                            skip_group_check=True)
                    else:
                        # K=96 composite: rows 0-31 Wih (on cx), 32-63 zero
                        # (Chat ignored), 64-95 0.5*Whh (on Hhat)
                        for c in range(3):
                            nc.tensor.matmul(
                                gp[32 * c:32 * c + 32, :],
                                wp[0:96, WIHG_OFF + 32 * c:WIHG_OFF + 32 * c + 32],
                                slprev[0:96, :], start=True, stop=True,
                                tile_position=(0, 32 * c),
                                skip_group_check=True)
                        nc.tensor.matmul(
                            gp_g, wp[0:96, WIHG_OFF + 96:WIHG_OFF + 128],
                            slprev[0:96, :], start=True, stop=True,
                            tile_position=(0, 0), skip_group_check=True)
                    tt = sb_step.tile([96, n], BF, tag="tt")
                    nc.scalar.activation(out=tt, in_=gp, func=AF.Tanh,
                                         scale=fp[0:96, SV_OFF:SV_OFF + 1],
                                         bias=fp[0:96, BT_OFF:BT_OFF + 1])
                    gt = sb_small.tile([32, n], BF, tag="gt")
                    nc.scalar.activation(out=gt, in_=gp_g, func=AF.Tanh,
                                         bias=fp[0:32, BG_OFF:BG_OFF + 1])
                    # ---- LSTM elementwise (bands: m1@0, m2@1) ----
                    mm = sb_step.tile([64, n], BF, tag="mm")
                    nc.vector.scalar_tensor_tensor(
                        out=mm[0:32, :], in0=tt[0:32, :], scalar=1.0, in1=gt,
                        op0=ALU.add, op1=ALU.mult)          # (1+ti)*g~
                    if s > 1:
                        nc.vector.scalar_tensor_tensor(
                            out=mm[32:64, :], in0=tt[32:64, :], scalar=1.0,
                            in1=slprev[32:64, :], op0=ALU.add, op1=ALU.mult)
                    # Chat' = m1 + 0.5*m2 -> bands 1 (carry) and 2 (tanh src)
                    cc_p = ps_sm.tile([96, n], F32, tag="psm")
                    for c in (1, 2):
                        if s == 1:
                            nc.tensor.matmul(
                                cc_p[32 * c:32 * c + 32, :],
                                wp[0:32, I32_OFF:I32_OFF + 32], mm[0:32, :],
                                start=True, stop=True,
                                tile_position=(0, 32 * c),
                                skip_group_check=True)
                        else:
                            nc.tensor.matmul(
                                cc_p[32 * c:32 * c + 32, :],
                                wp[0:64, CC_OFF:CC_OFF + 32], mm[0:64, :],
                                start=True, stop=True,
                                tile_position=(0, 32 * c),
                                skip_group_check=True)
                    sl = stash[:, (cj * FT + (s - 1)) * n:(cj * FT + s) * n]
                    nc.scalar.copy(out=sl[32:64, :], in_=cc_p[32:64, :])
                    tc_t = sb_small.tile([96, n], BF, tag="tct")
                    nc.scalar.activation(out=tc_t[64:96, :], in_=cc_p[64:96, :],
                                         func=AF.Tanh, scale=0.5)
                    nc.vector.scalar_tensor_tensor(
                        out=sl[64:96, :], in0=tt[64:96, :], scalar=1.0,
                        in1=tc_t[64:96, :],
                        op0=ALU.add, op1=ALU.mult)          # (1+to)*tanh(c)
                    hprev = sl[64:96, :]
                    slprev = sl

            # ---------------- phase B ----------------
            for cj in range(gsz):
                ci = g0 + cj
                nsub = n // 128
                lg_p = ps_sm.tile([96, n], F32, tag="psm")
                for s3 in range(FT):
                    nc.tensor.matmul(
                        lg_p[32 * s3:32 * s3 + 32, :],
                        wp[64:96, WOUT_OFF:WOUT_OFF + 32],
                        stash[64:96, (cj * FT + s3) * n:(cj * FT + s3 + 1) * n],
                        start=True, stop=True, tile_position=(64, 32 * s3), skip_group_check=True)
                eo = sb_step.tile([96, n], BF, tag="eo")
                nc.scalar.activation(out=eo, in_=lg_p, func=AF.Exp,
                                     bias=fp[0:96, BOUT_OFF:BOUT_OFF + 1])
                lgs = sb_step.tile([96, n], F32, tag="lgs")
                nc.scalar.activation(out=lgs, in_=lg_p, func=AF.Identity,
                                     bias=fp[0:96, BOUT_OFF:BOUT_OFF + 1])
                so_p = ps_sm.tile([96, n], F32, tag="psm")
                for s3 in range(FT):
                    nc.tensor.matmul(
                        so_p[32 * s3:32 * s3 + 32, :],
                        wp[32 * s3:32 * s3 + 32, ONES_OFF:ONES_OFF + 32],
                        eo[32 * s3:32 * s3 + 32, :],
                        start=True, stop=True,
                        tile_position=(32 * s3, 32 * s3), skip_group_check=True)
                ls = sb_step.tile([96, n], F32, tag="ls")
                nc.scalar.activation(out=ls, in_=so_p, func=AF.Ln)
                res = sb_step.tile([96, n], F32, tag="res")
                nc.gpsimd.tensor_sub(out=res, in0=lgs, in1=ls)
                # transpose to batch-major and write out
                ot_p = ps_sm.tile([128, nsub * 96], F32, tag="psm")
                for i in range(nsub):
                    nc.tensor.transpose(
                        ot_p[:, 96 * i:96 * i + 96],
                        res[:, 128 * i:128 * i + 128],
                        fp[0:96, IDT_OFF:IDT_OFF + 96])
                ob = sb_out.tile([128, nsub * 96], F32, tag="ob")
                nc.scalar.copy(out=ob, in_=ot_p)
                ob4 = ob.rearrange("p (i s o) -> p i s o", s=FT, o=32)
                for s3 in range(FT):
                    ov = out_d[ci * n:(ci + 1) * n, s3, :].rearrange(
                        "(i p) o -> p i o", p=128)
                    nc.sync.dma_start(out=ov, in_=ob4[:, :, s3, 0:OD])


_PROGRAM_CACHE: dict[int, bass.Bass] = {}
_LAST_EXEC_NS = None
_LAST_RESULTS = None


def _get_program(Bshard: int) -> bass.Bass:
    if Bshard not in _PROGRAM_CACHE:
        _PROGRAM_CACHE[Bshard] = build_program(Bshard)
    return _PROGRAM_CACHE[Bshard]


def kernel(**inputs) -> np.ndarray:
    import ml_dtypes
    h_enc = np.asarray(inputs["h_enc"], np.float32)
    B = h_enc.shape[0]
    Bshard = B // N_CORES
    wp, fpk = _pack_weights(
        inputs["Wq"], inputs["bq"], inputs["Wk"], inputs["bk"],
        inputs["Wv"], inputs["bv"], inputs["W_ih"], inputs["b_ih"],
        inputs["W_hh"], inputs["b_hh"], inputs["W_out"], inputs["b_out"])
    wp_bf = wp.astype(ml_dtypes.bfloat16)
    nc = _get_program(Bshard)
    in_maps = []
    for c in range(N_CORES):
        in_maps.append({
            "h_enc": np.ascontiguousarray(h_enc[c * Bshard:(c + 1) * Bshard]),
            "wpack": wp_bf,
            "fpack": fpk,
        })
    import os
    trace = bool(os.environ.get("BASS_TRACE"))
    res = run_bass_kernel_spmd(nc, in_maps, list(range(N_CORES)), trace=trace)
    global _LAST_EXEC_NS, _LAST_RESULTS
    _LAST_EXEC_NS = res.exec_time_ns
    _LAST_RESULTS = res
    outs = [np.asarray(res.results[c]["out"]).reshape(Bshard, FT, OD)
            for c in range(N_CORES)]
    return np.concatenate(outs, axis=0).astype(np.float32)



# revision 4
# speedup vs baseline: 3.0879x; 3.0879x over previous
"""Trainium2 Bass kernel v2 for nn_Decoder: attention+LSTM decoder.

Math (reference):
  k = h_enc @ Wk.T + bk ; v = h_enc @ Wv.T + bv        [B, 8, 32]
  3 decoder steps: q = h @ Wq.T + bq
     score_t = q.k_t/sqrt(32) ; att = softmax_t
     ctx = sum_t att_t v_t ; (h, c) = LSTMCell(ctx, h, c)
  logits_s = h_s @ Wout.T + b_out ; out = log_softmax(logits)   [B, 3, 10]

v2 algebra (host-side folds):
  score_t = rho . x_t  with  rho = 0.5*A.T Hhat + w,
     A = (Wq.T Wk)/sqrt(H), w = (Wk.T bq)/sqrt(H)   (t-indep terms dropped)
  ctx = Wv xbar + bv with xbar = sum_t att_t x_t  -> k and v never computed.
  gates = M1 xbar + 0.5*W_hh Hhat + bg,  M1 = W_ih@Wv, bg = b_ih+b_hh+W_ih@bv
  sigmoid via tanh; factor-2 carries Chat=2c, Hhat=2h.

Layout: feature-major t-packed tiles [128, n]: partition = 32*(t%4)+h,
halves lo (t0-3) / hi (t4-7) at free offsets 0/n in one [128, 2n] tile.
Scores kept COMPACT: per quad of 4 chunks, scq[32*j + t, :] = score_t of
chunk j (rows t<8 of band j); one exp per quad-step instead of 8.
LSTM state packed per quad: cq/hq [128, n] band j = chunk j's Chat/Hhat.
Gates per role (i,f,o,g) packed per quad -> 1 tanh per role per quad-step.
"""

import numpy as np

import concourse.bass as bass
import concourse.bacc as bacc
import concourse.tile as tile
from concourse import mybir
from concourse.bass_utils import run_bass_kernel_spmd

H = 32
HT = 8
FT = 3
OD = 10
N_CORES = 8

BF = mybir.dt.bfloat16
F32 = mybir.dt.float32
AF = mybir.ActivationFunctionType
ALU = mybir.AluOpType

CHUNK = 512          # batch elements per chunk
QUAD = 4             # chunks per packed state quad
GROUP = 16           # chunks per group (phase A/B batching, >=QUAD, %QUAD==0)

# wpack (bf16, [128, WCOLS]) column layout
ID128 = 0            # 128: identity(128) for input transposes
RHO4 = 128           # 128: 0.5*[A,A,A,A] rows replicated per band
CMPT = 256           # 2*32: compact ones-reduce, half h at CMPT+32h
CMPW = 320           # 2*32: compact w-reduce
EXPB = 384           # 2*128: e-broadcast, half h at EXPB+128h
SUMB = 640           # 32: ssum (rows<8 ones, broadcast out)
I32S = 672           # 32: eye(32) per band (t-reduce for ubar)
GX = 704             # 4*32: (M1 gate-c rows).T per band, c in {i,f,o,g}
GH = 832             # 4*32: (0.5*W_hh gate-c rows).T per band
LGW = 960            # 3*32: logits lhsT per step s (cols 10s+o)
BO10 = 1056          # 32: block-ones(10) for softmax sums
WCOLS = 1088

# fpack (f32, [128, FCOLS])
BIASC = 0            # 4 cols: gate ACT bias per role c
BOUTC = 4            # 1 col: b_out pattern (rows 32j+10s+o), -30 on pad rows
IDT128 = 5           # 128: f32 eye(128) (output transposes)
FCOLS = 133

GATE_SL = (slice(0, 32), slice(32, 64), slice(96, 128), slice(64, 96))  # i,f,o,g


def _pack_weights(Wq, bq, Wk, bk, Wv, bv, W_ih, b_ih, W_hh, b_hh, W_out, b_out):
    Wq, bq, Wk, bk, Wv, bv, W_ih, b_ih, W_hh, b_hh, W_out, b_out = [
        np.asarray(a, np.float32) for a in
        (Wq, bq, Wk, bk, Wv, bv, W_ih, b_ih, W_hh, b_hh, W_out, b_out)]
    s = 1.0 / np.sqrt(np.float32(H))
    A = (Wq.T @ Wk) * s                    # [32,32]
    w = (Wk.T @ bq) * s                    # [32]
    M1 = W_ih @ Wv                         # [128,32]
    bg = b_ih + b_hh + W_ih @ bv           # [128] in (i,f,g,o) order

    wp = np.zeros((128, WCOLS), np.float32)
    wp[:, ID128:ID128 + 128] = np.eye(128)
    for r in range(4):
        P = slice(32 * r, 32 * r + 32)
        for c in range(4):
            wp[P, RHO4 + 32 * c:RHO4 + 32 * c + 32] = 0.5 * A
        for hf in range(2):
            wp[P, CMPT + 32 * hf + (4 * hf + r)] = 1.0
            wp[P, CMPW + 32 * hf + (4 * hf + r)] = w
            for c in range(4):
                wp[32 * r + (4 * hf + c),
                   EXPB + 128 * hf + 32 * c:EXPB + 128 * hf + 32 * c + 32] = 1.0
        wp[32 * r:32 * r + 8, SUMB:SUMB + 32] = 1.0
        wp[P, I32S:I32S + 32] = np.eye(32)
        for c, gsl in enumerate(GATE_SL):
            wp[P, GX + 32 * c:GX + 32 * c + 32] = M1[gsl].T
            wp[P, GH + 32 * c:GH + 32 * c + 32] = (0.5 * W_hh[gsl]).T
        for st in range(FT):
            for o in range(OD):
                wp[P, LGW + 32 * st + OD * st + o] = 0.5 * W_out[o]
        bo = np.zeros((32, 32), np.float32)
        for kk in range(30):
            for oo in range(30):
                if kk // OD == oo // OD:
                    bo[kk, oo] = 1.0
        bo[30, 30] = 1.0
        bo[31, 31] = 1.0
        wp[P, BO10:BO10 + 32] = bo

    fp = np.zeros((128, FCOLS), np.float32)
    for r in range(4):
        P = slice(32 * r, 32 * r + 32)
        for c in range(3):
            fp[P, BIASC + c] = 0.5 * bg[GATE_SL[c]]
        fp[P, BIASC + 3] = bg[GATE_SL[3]]
        bout = np.full(32, -30.0, np.float32)
        for st in range(FT):
            bout[OD * st:OD * st + OD] = b_out
        fp[P, BOUTC] = bout
    fp[:, IDT128:IDT128 + 128] = np.eye(128)
    return wp, fp


def build_program(Bshard: int) -> bass.Bass:
    assert Bshard % (QUAD * CHUNK) == 0
    nchunks = Bshard // CHUNK
    nc = bacc.Bacc(trn_type="TRN2")
    x_d = nc.declare_dram_parameter("h_enc", [Bshard, HT, H], F32, isOutput=False)
    wp_d = nc.declare_dram_parameter("wpack", [128, WCOLS], BF, isOutput=False)
    fp_d = nc.declare_dram_parameter("fpack", [128, FCOLS], F32, isOutput=False)
    out_d = nc.declare_dram_parameter("out", [Bshard, FT, OD], F32, isOutput=True)
    with tile.TileContext(nc) as tc:
        _body(nc, tc, x_d, wp_d, fp_d, out_d, nchunks, CHUNK)
    _split_matmul_waits(nc)
    nc.compile()
    return nc


def _split_matmul_waits(nc):
    """Walrus instruction structs fit one sync wait; move extras onto
    same-engine no-ops (each carrying a single wait) inserted just before."""
    for b in nc.m.functions[0].blocks:
        new = []
        for ins in b.instructions:
            si = ins.sync_info
            if (si is not None and len(si.on_wait) > 1
                    and not isinstance(ins, (mybir.InstEventSemaphore,
                                             mybir.InstNoOp))):
                for w in si.on_wait[:-1]:
                    nop = mybir.InstNoOp(
                        name=nc.get_next_instruction_name(), ins=[], outs=[],
                        engine=ins.engine,
                        sync_info=mybir.SyncInfo(on_wait=[w], on_update=[]))
                    nc.register_instruction(nop)
                    new.append(nop)
                ins.sync_info = mybir.SyncInfo(
                    on_wait=[si.on_wait[-1]], on_update=list(si.on_update))
            new.append(ins)
        b.instructions[:] = new


def _body(nc, tc, x_d, wp_d, fp_d, out_d, nchunks, n):
    from contextlib import ExitStack
    ctx = ExitStack()
    with ctx:
        singles = ctx.enter_context(tc.tile_pool(name="singles", bufs=1))
        sb_xb = ctx.enter_context(tc.tile_pool(name="sb_xb", bufs=3))
        sb_xt = ctx.enter_context(tc.tile_pool(name="sb_xt", bufs=GROUP + 2))
        sb_e = ctx.enter_context(tc.tile_pool(name="sb_e", bufs=3))
        sb_at = ctx.enter_context(tc.tile_pool(name="sb_at", bufs=6))
        sb_m = ctx.enter_context(tc.tile_pool(name="sb_m", bufs=2))
        sb_rs = ctx.enter_context(tc.tile_pool(name="sb_rs", bufs=2))
        sb_xq = ctx.enter_context(tc.tile_pool(name="sb_xq", bufs=2))
        sb_tg = ctx.enter_context(tc.tile_pool(name="sb_tg", bufs=2))
        sb_cq = ctx.enter_context(tc.tile_pool(name="sb_cq", bufs=GROUP // QUAD + 2))
        sb_hq = ctx.enter_context(tc.tile_pool(name="sb_hq", bufs=3 * (GROUP // QUAD) + 2))
        sb_ph = ctx.enter_context(tc.tile_pool(name="sb_ph", bufs=GROUP // QUAD + 1))
        sb_oc = ctx.enter_context(tc.tile_pool(name="sb_oc", bufs=2))
        ps_x = ctx.enter_context(tc.tile_pool(name="ps_x", bufs=2, space="PSUM"))
        ps_f = ctx.enter_context(tc.tile_pool(name="ps_f", bufs=6, space="PSUM"))

        wp = singles.tile([128, WCOLS], BF)
        nc.sync.dma_start(out=wp, in_=wp_d[:, :])
        fp = singles.tile([128, FCOLS], F32)
        nc.sync.dma_start(out=fp, in_=fp_d[:, :])

        ident = wp[:, ID128:ID128 + 128]
        nquads = GROUP // QUAD

        g0 = 0
        while g0 < nchunks:
            gsz = min(GROUP, nchunks - g0)
            gq = gsz // QUAD
            # ---------------- phase A: load + transpose ----------------
            xts = []
            for cj in range(gsz):
                ci = g0 + cj
                xb = sb_xb.tile([128, 4, 256], BF, tag="xb")
                xv = x_d[ci * n:(ci + 1) * n].rearrange(
                    "(i p) t h -> p i (t h)", p=128)
                nc.gpsimd.dma_start(out=xb, in_=xv)
                xp = ps_x.tile([128, 2 * n], BF, tag="xp")
                for hf in range(2):
                    for i in range(4):
                        nc.tensor.transpose(
                            xp[:, n * hf + 128 * i:n * hf + 128 * i + 128],
                            xb[:, i, 128 * hf:128 * hf + 128],
                            ident)
                xt = sb_xt.tile([128, 2 * n], BF, tag="xt")
                nc.vector.tensor_copy(
                    xt[:, :].bitcast(mybir.dt.int32),
                    xp[:, :].bitcast(mybir.dt.int32))
                xts.append(xt)

            # ---------------- recurrent steps ----------------
            hq_all = {}
            cq_prev = {}
            for s in range(1, FT + 1):
                for q in range(gq):
                    jj = [q * QUAD + j for j in range(QUAD)]
                    hq_prev = hq_all.get((q, s - 1))

                    pt = {}
                    if s > 1:
                        for j in range(QUAD):
                            rp = ps_f.tile([128, n], F32, tag="f")
                            nc.tensor.matmul(
                                rp[:, :], wp[32 * j:32 * j + 32, RHO4:RHO4 + 128],
                                hq_prev[32 * j:32 * j + 32, :],
                                start=True, stop=True,
                                tile_position=(32 * j, 0), skip_group_check=True)
                            for hf in range(2):
                                t = sb_at.tile([128, n], BF, tag="pt")
                                nc.vector.tensor_tensor(
                                    out=t, in0=rp,
                                    in1=xts[jj[j]][:, n * hf:n * hf + n],
                                    op=ALU.mult)
                                pt[(j, hf)] = t

                    scq = ps_f.tile([128, n], F32, tag="f")
                    for j in range(QUAD):
                        mms = []
                        if s > 1:
                            for hf in range(2):
                                mms.append((wp[0:128, CMPT + 32 * hf:CMPT + 32 * hf + 32],
                                            pt[(j, hf)][:, :]))
                        for hf in range(2):
                            mms.append((wp[0:128, CMPW + 32 * hf:CMPW + 32 * hf + 32],
                                        xts[jj[j]][:, n * hf:n * hf + n]))
                        for k, (lh, rh) in enumerate(mms):
                            nc.tensor.matmul(
                                scq[32 * j:32 * j + 32, :], lh, rh,
                                start=(k == 0), stop=(k == len(mms) - 1),
                                tile_position=(0, 32 * j), skip_group_check=True)

                    esc = sb_e.tile([128, n], BF, tag="esc")
                    nc.scalar.activation(out=esc, in_=scq, func=AF.Exp)

                    ssq = ps_f.tile([128, n], F32, tag="f")
                    for j in range(QUAD):
                        nc.tensor.matmul(
                            ssq[32 * j:32 * j + 32, :],
                            wp[32 * j:32 * j + 32, SUMB:SUMB + 32],
                            esc[32 * j:32 * j + 32, :],
                            start=True, stop=True,
                            tile_position=(32 * j, 32 * j), skip_group_check=True)
                    rs = sb_rs.tile([128, n], F32, tag="rs")
                    nc.vector.reciprocal_approx_fast(out=rs, in_=ssq)

                    at = {}
                    for hf in range(2):
                        for j in range(QUAD):
                            ebp = ps_f.tile([128, n], F32, tag="f")
                            nc.tensor.matmul(
                                ebp[:, :],
                                wp[32 * j:32 * j + 32, EXPB + 128 * hf:EXPB + 128 * hf + 128],
                                esc[32 * j:32 * j + 32, :],
                                start=True, stop=True,
                                tile_position=(32 * j, 0), skip_group_check=True)
                            t = sb_at.tile([128, n], BF, tag="at")
                            nc.vector.tensor_tensor(
                                out=t, in0=ebp,
                                in1=xts[jj[j]][:, n * hf:n * hf + n],
                                op=ALU.mult)
                            at[(j, hf)] = t

                    ubq = ps_f.tile([128, n], F32, tag="f")
                    for j in range(QUAD):
                        for hf in range(2):
                            nc.tensor.matmul(
                                ubq[32 * j:32 * j + 32, :],
                                wp[0:128, I32S:I32S + 32], at[(j, hf)][:, :],
                                start=(hf == 0), stop=(hf == 1),
                                tile_position=(0, 32 * j), skip_group_check=True)
                    xq = sb_xq.tile([128, n], BF, tag="xq")
                    nc.vector.tensor_tensor(out=xq, in0=ubq, in1=rs, op=ALU.mult)

                    tg = []
                    for c in range(4):
                        gp = ps_f.tile([128, n], F32, tag="f")
                        for j in range(QUAD):
                            nc.tensor.matmul(
                                gp[32 * j:32 * j + 32, :],
                                wp[32 * j:32 * j + 32, GX + 32 * c:GX + 32 * c + 32],
                                xq[32 * j:32 * j + 32, :],
                                start=True, stop=(s == 1),
                                tile_position=(32 * j, 32 * j), skip_group_check=True)
                        if s > 1:
                            for j in range(QUAD):
                                nc.tensor.matmul(
                                    gp[32 * j:32 * j + 32, :],
                                    wp[32 * j:32 * j + 32, GH + 32 * c:GH + 32 * c + 32],
                                    hq_prev[32 * j:32 * j + 32, :],
                                    start=False, stop=True,
                                    tile_position=(32 * j, 32 * j), skip_group_check=True)
                        t = sb_tg.tile([128, n], BF, tag=f"tg{c}")
                        if c < 3:
                            nc.scalar.activation(
                                out=t, in_=gp, func=AF.Tanh, scale=0.5,
                                bias=fp[:, BIASC + c:BIASC + c + 1])
                        else:
                            nc.scalar.activation(
                                out=t, in_=gp, func=AF.Tanh,
                                bias=fp[:, BIASC + 3:BIASC + 4])
                        tg.append(t)

                    cq_new = sb_cq.tile([128, n], BF, tag="cq")
                    if s == 1:
                        nc.vector.scalar_tensor_tensor(
                            out=cq_new, in0=tg[0], scalar=1.0, in1=tg[3],
                            op0=ALU.add, op1=ALU.mult)
                    else:
                        m1 = sb_m.tile([128, n], BF, tag="m1")
                        nc.vector.scalar_tensor_tensor(
                            out=m1, in0=tg[0], scalar=1.0, in1=tg[3],
                            op0=ALU.add, op1=ALU.mult)
                        u2 = sb_m.tile([128, n], BF, tag="u2")
                        nc.gpsimd.tensor_tensor(
                            out=u2, in0=tg[1], in1=cq_prev[q], op=ALU.mult)
                        m2 = sb_m.tile([128, n], BF, tag="m2")
                        nc.gpsimd.tensor_tensor(
                            out=m2, in0=u2, in1=cq_prev[q], op=ALU.add)
                        nc.vector.scalar_tensor_tensor(
                            out=cq_new, in0=m2, scalar=0.5, in1=m1,
                            op0=ALU.mult, op1=ALU.add)
                    cq_prev[q] = cq_new

                    tcq = sb_tg.tile([128, n], BF, tag="tcq")
                    nc.scalar.activation(out=tcq, in_=cq_new, func=AF.Tanh,
                                         scale=0.5)
                    hq = sb_hq.tile([128, n], BF, tag="hq")
                    nc.vector.scalar_tensor_tensor(
                        out=hq, in0=tg[2], scalar=1.0, in1=tcq,
                        op0=ALU.add, op1=ALU.mult)
                    hq_all[(q, s)] = hq

            # ---------------- phase B ----------------
            sums = []
            lgss = []
            for q in range(gq):
                lgq = ps_f.tile([128, n], F32, tag="f")
                for j in range(QUAD):
                    for s in range(1, FT + 1):
                        nc.tensor.matmul(
                            lgq[32 * j:32 * j + 32, :],
                            wp[32 * j:32 * j + 32, LGW + 32 * (s - 1):LGW + 32 * s],
                            hq_all[(q, s)][32 * j:32 * j + 32, :],
                            start=(s == 1), stop=(s == FT),
                            tile_position=(32 * j, 32 * j), skip_group_check=True)
                eoq = sb_e.tile([128, n], BF, tag="eoq")
                nc.scalar.activation(out=eoq, in_=lgq, func=AF.Exp,
                                     bias=fp[:, BOUTC:BOUTC + 1])
                lgs = sb_ph.tile([128, n], F32, tag="lgs")
                nc.scalar.activation(out=lgs, in_=lgq, func=AF.Identity,
                                     bias=fp[:, BOUTC:BOUTC + 1])
                lgss.append(lgs)
                soq = ps_f.tile([128, n], F32, tag="f")
                for j in range(QUAD):
                    nc.tensor.matmul(
                        soq[32 * j:32 * j + 32, :],
                        wp[32 * j:32 * j + 32, BO10:BO10 + 32],
                        eoq[32 * j:32 * j + 32, :],
                        start=True, stop=True,
                        tile_position=(32 * j, 32 * j), skip_group_check=True)
                sm = sb_ph.tile([128, n], F32, tag="sm")
                nc.vector.tensor_copy(sm, soq)
                sums.append(sm)

            for q in range(gq):
                ls = sb_ph.tile([128, n], F32, tag="ls")
                nc.scalar.activation(out=ls, in_=sums[q], func=AF.Ln)
                res = sb_ph.tile([128, n], F32, tag="res")
                nc.gpsimd.tensor_sub(out=res, in0=lgss[q], in1=ls)
                otp = ps_f.tile([128, 4 * 128], F32, tag="f")
                for bb in range(4):
                    nc.tensor.transpose(
                        otp[:, 128 * bb:128 * bb + 128],
                        res[:, 128 * bb:128 * bb + 128],
                        fp[:, IDT128:IDT128 + 128])
                oc = sb_oc.tile([128, 4 * 128], F32, tag="oc")
                nc.vector.tensor_copy(
                    oc.rearrange("p (j b so) -> p b j so", j=4, b=4),
                    otp.rearrange("p (b j so) -> p b j so", b=4, j=4))
                oc4 = oc.rearrange("p (j b so) -> p j b so", j=4, b=4)
                base = (g0 + q * QUAD) * n
                ov = out_d[base:base + QUAD * n, :, :].rearrange(
                    "(j b p) s o -> p j b (s o)", p=128, b=4)
                nc.sync.dma_start(out=ov, in_=oc4[:, :, :, 0:FT * OD])
            g0 += gsz


_PROGRAM_CACHE: dict[int, bass.Bass] = {}
_LAST_EXEC_NS = None
_LAST_RESULTS = None


def _get_program(Bshard: int) -> bass.Bass:
    if Bshard not in _PROGRAM_CACHE:
        _PROGRAM_CACHE[Bshard] = build_program(Bshard)
    return _PROGRAM_CACHE[Bshard]


def kernel(**inputs) -> np.ndarray:
    import ml_dtypes
    h_enc = np.asarray(inputs["h_enc"], np.float32)
    B = h_enc.shape[0]
    Bshard = B // N_CORES
    wp, fpk = _pack_weights(
        inputs["Wq"], inputs["bq"], inputs["Wk"], inputs["bk"],
        inputs["Wv"], inputs["bv"], inputs["W_ih"], inputs["b_ih"],
        inputs["W_hh"], inputs["b_hh"], inputs["W_out"], inputs["b_out"])
    wp_bf = wp.astype(ml_dtypes.bfloat16)
    nc = _get_program(Bshard)
    in_maps = []
    for c in range(N_CORES):
        in_maps.append({
            "h_enc": np.ascontiguousarray(h_enc[c * Bshard:(c + 1) * Bshard]),
            "wpack": wp_bf,
            "fpack": fpk,
        })
    import os
    trace = bool(os.environ.get("BASS_TRACE"))
    res = run_bass_kernel_spmd(nc, in_maps, list(range(N_CORES)), trace=trace)
    global _LAST_EXEC_NS, _LAST_RESULTS
    _LAST_EXEC_NS = res.exec_time_ns
    _LAST_RESULTS = res
    outs = [np.asarray(res.results[c]["out"]).reshape(Bshard, FT, OD)
            for c in range(N_CORES)]
    return np.concatenate(outs, axis=0).astype(np.float32)


# revision 5
# speedup vs baseline: 3.0888x; 1.0003x over previous
"""Trainium2 Bass kernel v2 for nn_Decoder: attention+LSTM decoder.

Math (reference):
  k = h_enc @ Wk.T + bk ; v = h_enc @ Wv.T + bv        [B, 8, 32]
  3 decoder steps: q = h @ Wq.T + bq
     score_t = q.k_t/sqrt(32) ; att = softmax_t
     ctx = sum_t att_t v_t ; (h, c) = LSTMCell(ctx, h, c)
  logits_s = h_s @ Wout.T + b_out ; out = log_softmax(logits)   [B, 3, 10]

v2 algebra (host-side folds):
  score_t = rho . x_t  with  rho = 0.5*A.T Hhat + w,
     A = (Wq.T Wk)/sqrt(H), w = (Wk.T bq)/sqrt(H)   (t-indep terms dropped)
  ctx = Wv xbar + bv with xbar = sum_t att_t x_t  -> k and v never computed.
  gates = M1 xbar + 0.5*W_hh Hhat + bg,  M1 = W_ih@Wv, bg = b_ih+b_hh+W_ih@bv
  sigmoid via tanh; factor-2 carries Chat=2c, Hhat=2h.

Layout: feature-major t-packed tiles [128, n]: partition = 32*(t%4)+h,
halves lo (t0-3) / hi (t4-7) at free offsets 0/n in one [128, 2n] tile.
Scores kept COMPACT: per quad of 4 chunks, scq[32*j + t, :] = score_t of
chunk j (rows t<8 of band j); one exp per quad-step instead of 8.
LSTM state packed per quad: cq/hq [128, n] band j = chunk j's Chat/Hhat.
Gates per role (i,f,o,g) packed per quad -> 1 tanh per role per quad-step.
"""

import numpy as np

import concourse.bass as bass
import concourse.bacc as bacc
import concourse.tile as tile
from concourse import mybir
from concourse.bass_utils import run_bass_kernel_spmd

H = 32
HT = 8
FT = 3
OD = 10
N_CORES = 8

BF = mybir.dt.bfloat16
F32 = mybir.dt.float32
AF = mybir.ActivationFunctionType
ALU = mybir.AluOpType

CHUNK = 512          # batch elements per chunk
QUAD = 4             # chunks per packed state quad
GROUP = 16           # chunks per group (phase A/B batching, >=QUAD, %QUAD==0)

# wpack (bf16, [128, WCOLS]) column layout
ID128 = 0            # 128: identity(128) for input transposes
RHO4 = 128           # 128: 0.5*[A,A,A,A] rows replicated per band
CMPT = 256           # 2*32: compact ones-reduce, half h at CMPT+32h
CMPW = 320           # 2*32: compact w-reduce
EXPB = 384           # 2*128: e-broadcast, half h at EXPB+128h
SUMB = 640           # 32: ssum (rows<8 ones, broadcast out)
I32S = 672           # 32: eye(32) per band (t-reduce for ubar)
GX = 704             # 4*32: (M1 gate-c rows).T per band, c in {i,f,o,g}
GH = 832             # 4*32: (0.5*W_hh gate-c rows).T per band
LGW = 960            # 3*32: logits lhsT per step s (cols 10s+o)
BO10 = 1056          # 32: block-ones(10) for softmax sums
WCOLS = 1088

# fpack (f32, [128, FCOLS])
BIASC = 0            # 4 cols: gate ACT bias per role c
BOUTC = 4            # 1 col: b_out pattern (rows 32j+10s+o), -30 on pad rows
WVEC = 5             # 1 col: w replicated per band (pt STT scalar)
IDT128 = 6           # 128: f32 eye(128) (output transposes)
FCOLS = 134

GATE_SL = (slice(0, 32), slice(32, 64), slice(96, 128), slice(64, 96))  # i,f,o,g


def _pack_weights(Wq, bq, Wk, bk, Wv, bv, W_ih, b_ih, W_hh, b_hh, W_out, b_out):
    Wq, bq, Wk, bk, Wv, bv, W_ih, b_ih, W_hh, b_hh, W_out, b_out = [
        np.asarray(a, np.float32) for a in
        (Wq, bq, Wk, bk, Wv, bv, W_ih, b_ih, W_hh, b_hh, W_out, b_out)]
    s = 1.0 / np.sqrt(np.float32(H))
    A = (Wq.T @ Wk) * s                    # [32,32]
    w = (Wk.T @ bq) * s                    # [32]
    M1 = W_ih @ Wv                         # [128,32]
    bg = b_ih + b_hh + W_ih @ bv           # [128] in (i,f,g,o) order

    wp = np.zeros((128, WCOLS), np.float32)
    wp[:, ID128:ID128 + 128] = np.eye(128)
    for r in range(4):
        P = slice(32 * r, 32 * r + 32)
        for c in range(4):
            wp[P, RHO4 + 32 * c:RHO4 + 32 * c + 32] = 0.5 * A
        for hf in range(2):
            wp[P, CMPT + 32 * hf + (4 * hf + r)] = 1.0
            wp[P, CMPW + 32 * hf + (4 * hf + r)] = w
            for c in range(4):
                wp[32 * r + (4 * hf + c),
                   EXPB + 128 * hf + 32 * c:EXPB + 128 * hf + 32 * c + 32] = 1.0
        wp[32 * r:32 * r + 8, SUMB:SUMB + 32] = 1.0
        wp[P, I32S:I32S + 32] = np.eye(32)
        for c, gsl in enumerate(GATE_SL):
            wp[P, GX + 32 * c:GX + 32 * c + 32] = M1[gsl].T
            wp[P, GH + 32 * c:GH + 32 * c + 32] = (0.5 * W_hh[gsl]).T
        for st in range(FT):
            for o in range(OD):
                wp[P, LGW + 32 * st + OD * st + o] = 0.5 * W_out[o]
        bo = np.zeros((32, 32), np.float32)
        for kk in range(30):
            for oo in range(30):
                if kk // OD == oo // OD:
                    bo[kk, oo] = 1.0
        bo[30, 30] = 1.0
        bo[31, 31] = 1.0
        wp[P, BO10:BO10 + 32] = bo

    fp = np.zeros((128, FCOLS), np.float32)
    for r in range(4):
        P = slice(32 * r, 32 * r + 32)
        for c in range(3):
            fp[P, BIASC + c] = 0.5 * bg[GATE_SL[c]]
        fp[P, BIASC + 3] = bg[GATE_SL[3]]
        bout = np.full(32, -30.0, np.float32)
        for st in range(FT):
            bout[OD * st:OD * st + OD] = b_out
        fp[P, BOUTC] = bout
        fp[P, WVEC] = w
    fp[:, IDT128:IDT128 + 128] = np.eye(128)
    return wp, fp


def build_program(Bshard: int) -> bass.Bass:
    assert Bshard % (QUAD * CHUNK) == 0
    nchunks = Bshard // CHUNK
    nc = bacc.Bacc(trn_type="TRN2")
    x_d = nc.declare_dram_parameter("h_enc", [Bshard, HT, H], F32, isOutput=False)
    wp_d = nc.declare_dram_parameter("wpack", [128, WCOLS], BF, isOutput=False)
    fp_d = nc.declare_dram_parameter("fpack", [128, FCOLS], F32, isOutput=False)
    out_d = nc.declare_dram_parameter("out", [Bshard, FT, OD], F32, isOutput=True)
    with tile.TileContext(nc) as tc:
        _body(nc, tc, x_d, wp_d, fp_d, out_d, nchunks, CHUNK)
    _split_matmul_waits(nc)
    nc.compile()
    return nc


def _split_matmul_waits(nc):
    """Walrus instruction structs fit one sync wait; move extras onto
    same-engine no-ops (each carrying a single wait) inserted just before."""
    for b in nc.m.functions[0].blocks:
        new = []
        for ins in b.instructions:
            si = ins.sync_info
            if (si is not None and len(si.on_wait) > 1
                    and not isinstance(ins, (mybir.InstEventSemaphore,
                                             mybir.InstNoOp))):
                for w in si.on_wait[:-1]:
                    nop = mybir.InstNoOp(
                        name=nc.get_next_instruction_name(), ins=[], outs=[],
                        engine=ins.engine,
                        sync_info=mybir.SyncInfo(on_wait=[w], on_update=[]))
                    nc.register_instruction(nop)
                    new.append(nop)
                ins.sync_info = mybir.SyncInfo(
                    on_wait=[si.on_wait[-1]], on_update=list(si.on_update))
            new.append(ins)
        b.instructions[:] = new


def _body(nc, tc, x_d, wp_d, fp_d, out_d, nchunks, n):
    from contextlib import ExitStack
    ctx = ExitStack()
    with ctx:
        singles = ctx.enter_context(tc.tile_pool(name="singles", bufs=1))
        sb_xb = ctx.enter_context(tc.tile_pool(name="sb_xb", bufs=3))
        sb_xt = ctx.enter_context(tc.tile_pool(name="sb_xt", bufs=GROUP + 2))
        sb_e = ctx.enter_context(tc.tile_pool(name="sb_e", bufs=3))
        sb_at = ctx.enter_context(tc.tile_pool(name="sb_at", bufs=6))
        sb_m = ctx.enter_context(tc.tile_pool(name="sb_m", bufs=2))
        sb_rs = ctx.enter_context(tc.tile_pool(name="sb_rs", bufs=2))
        sb_xq = ctx.enter_context(tc.tile_pool(name="sb_xq", bufs=2))
        sb_tg = ctx.enter_context(tc.tile_pool(name="sb_tg", bufs=2))
        sb_cq = ctx.enter_context(tc.tile_pool(name="sb_cq", bufs=GROUP // QUAD + 2))
        sb_hq = ctx.enter_context(tc.tile_pool(name="sb_hq", bufs=3 * (GROUP // QUAD) + 2))
        sb_ph = ctx.enter_context(tc.tile_pool(name="sb_ph", bufs=GROUP // QUAD + 1))
        sb_oc = ctx.enter_context(tc.tile_pool(name="sb_oc", bufs=2))
        ps_x = ctx.enter_context(tc.tile_pool(name="ps_x", bufs=2, space="PSUM"))
        ps_f = ctx.enter_context(tc.tile_pool(name="ps_f", bufs=6, space="PSUM"))

        wp = singles.tile([128, WCOLS], BF)
        nc.sync.dma_start(out=wp, in_=wp_d[:, :])
        fp = singles.tile([128, FCOLS], F32)
        nc.sync.dma_start(out=fp, in_=fp_d[:, :])

        ident = wp[:, ID128:ID128 + 128]
        nquads = GROUP // QUAD

        g0 = 0
        while g0 < nchunks:
            gsz = min(GROUP, nchunks - g0)
            gq = gsz // QUAD
            # ---------------- phase A: load + transpose ----------------
            xts = []
            for cj in range(gsz):
                ci = g0 + cj
                xb = sb_xb.tile([128, 4, 256], BF, tag="xb")
                xv = x_d[ci * n:(ci + 1) * n].rearrange(
                    "(i p) t h -> p i (t h)", p=128)
                nc.gpsimd.dma_start(out=xb, in_=xv)
                xp = ps_x.tile([128, 2 * n], BF, tag="xp")
                for hf in range(2):
                    for i in range(4):
                        nc.tensor.transpose(
                            xp[:, n * hf + 128 * i:n * hf + 128 * i + 128],
                            xb[:, i, 128 * hf:128 * hf + 128],
                            ident)
                xt = sb_xt.tile([128, 2 * n], BF, tag="xt")
                nc.vector.tensor_copy(
                    xt[:, :].bitcast(mybir.dt.int32),
                    xp[:, :].bitcast(mybir.dt.int32))
                xts.append(xt)

            # ---------------- recurrent steps ----------------
            hq_all = {}
            cq_prev = {}
            for s in range(1, FT + 1):
                for q in range(gq):
                    jj = [q * QUAD + j for j in range(QUAD)]
                    hq_prev = hq_all.get((q, s - 1))

                    pt = {}
                    if s > 1:
                        rps = []
                        for j in range(QUAD):
                            rp = ps_f.tile([128, n], F32, tag="f")
                            nc.tensor.matmul(
                                rp[:, :], wp[32 * j:32 * j + 32, RHO4:RHO4 + 128],
                                hq_prev[32 * j:32 * j + 32, :],
                                start=True, stop=True,
                                tile_position=(32 * j, 0), skip_group_check=True)
                            rps.append(rp)
                        for j in range(QUAD):
                            for hf in range(2):
                                t = sb_at.tile([128, n], BF, tag="pt")
                                nc.vector.scalar_tensor_tensor(
                                    out=t, in0=rps[j],
                                    scalar=fp[:, WVEC:WVEC + 1],
                                    in1=xts[jj[j]][:, n * hf:n * hf + n],
                                    op0=ALU.add, op1=ALU.mult)
                                pt[(j, hf)] = t

                    scq = ps_f.tile([128, n], F32, tag="f")
                    if s > 1:
                        mm_of = lambda j, k: (
                            wp[0:128, CMPT + 32 * k:CMPT + 32 * k + 32],
                            pt[(j, k)][:, :])
                        nmm = 2
                    else:
                        mm_of = lambda j, k: (
                            wp[0:128, CMPW + 32 * k:CMPW + 32 * k + 32],
                            xts[jj[j]][:, n * k:n * k + n])
                        nmm = 2
                    for k in range(nmm):
                        for j in range(QUAD):
                            lh, rh = mm_of(j, k)
                            nc.tensor.matmul(
                                scq[32 * j:32 * j + 32, :], lh, rh,
                                start=(k == 0), stop=(k == nmm - 1),
                                tile_position=(0, 32 * j), skip_group_check=True)

                    esc = sb_e.tile([128, n], BF, tag="esc")
                    nc.scalar.activation(out=esc, in_=scq, func=AF.Exp)

                    ssq = ps_f.tile([128, n], F32, tag="f")
                    for j in range(QUAD):
                        nc.tensor.matmul(
                            ssq[32 * j:32 * j + 32, :],
                            wp[32 * j:32 * j + 32, SUMB:SUMB + 32],
                            esc[32 * j:32 * j + 32, :],
                            start=True, stop=True,
                            tile_position=(32 * j, 32 * j), skip_group_check=True)
                    rs = sb_rs.tile([128, n], F32, tag="rs")
                    nc.vector.reciprocal_approx_fast(out=rs, in_=ssq)

                    at = {}
                    for hf in range(2):
                        for j in range(QUAD):
                            ebp = ps_f.tile([128, n], F32, tag="f")
                            nc.tensor.matmul(
                                ebp[:, :],
                                wp[32 * j:32 * j + 32, EXPB + 128 * hf:EXPB + 128 * hf + 128],
                                esc[32 * j:32 * j + 32, :],
                                start=True, stop=True,
                                tile_position=(32 * j, 0), skip_group_check=True)
                            t = sb_at.tile([128, n], BF, tag="at")
                            nc.vector.tensor_tensor(
                                out=t, in0=ebp,
                                in1=xts[jj[j]][:, n * hf:n * hf + n],
                                op=ALU.mult)
                            at[(j, hf)] = t

                    ubq = ps_f.tile([128, n], F32, tag="f")
                    for hf in range(2):
                        for j in range(QUAD):
                            nc.tensor.matmul(
                                ubq[32 * j:32 * j + 32, :],
                                wp[0:128, I32S:I32S + 32], at[(j, hf)][:, :],
                                start=(hf == 0), stop=(hf == 1),
                                tile_position=(0, 32 * j), skip_group_check=True)
                    xq = sb_xq.tile([128, n], BF, tag="xq")
                    nc.vector.tensor_tensor(out=xq, in0=ubq, in1=rs, op=ALU.mult)

                    gps = []
                    for _gi in range(4):
                        gp_t = ps_f.tile([128, n], F32, tag="f")
                        gps.append(gp_t)
                    for c in range(4):
                        for j in range(QUAD):
                            nc.tensor.matmul(
                                gps[c][32 * j:32 * j + 32, :],
                                wp[32 * j:32 * j + 32, GX + 32 * c:GX + 32 * c + 32],
                                xq[32 * j:32 * j + 32, :],
                                start=True, stop=(s == 1),
                                tile_position=(32 * j, 32 * j), skip_group_check=True)
                    if s > 1:
                        for c in range(4):
                            for j in range(QUAD):
                                nc.tensor.matmul(
                                    gps[c][32 * j:32 * j + 32, :],
                                    wp[32 * j:32 * j + 32, GH + 32 * c:GH + 32 * c + 32],
                                    hq_prev[32 * j:32 * j + 32, :],
                                    start=False, stop=True,
                                    tile_position=(32 * j, 32 * j), skip_group_check=True)
                    tg = []
                    for c in range(4):
                        t = sb_tg.tile([128, n], BF, tag=f"tg{c}")
                        if c < 3:
                            nc.scalar.activation(
                                out=t, in_=gps[c], func=AF.Tanh, scale=0.5,
                                bias=fp[:, BIASC + c:BIASC + c + 1])
                        else:
                            nc.scalar.activation(
                                out=t, in_=gps[c], func=AF.Tanh,
                                bias=fp[:, BIASC + 3:BIASC + 4])
                        tg.append(t)

                    cq_new = sb_cq.tile([128, n], BF, tag="cq")
                    if s == 1:
                        nc.vector.scalar_tensor_tensor(
                            out=cq_new, in0=tg[0], scalar=1.0, in1=tg[3],
                            op0=ALU.add, op1=ALU.mult)
                    else:
                        m1 = sb_m.tile([128, n], BF, tag="m1")
                        nc.vector.scalar_tensor_tensor(
                            out=m1, in0=tg[0], scalar=1.0, in1=tg[3],
                            op0=ALU.add, op1=ALU.mult)
                        u2 = sb_m.tile([128, n], BF, tag="u2")
                        nc.gpsimd.tensor_tensor(
                            out=u2, in0=tg[1], in1=cq_prev[q], op=ALU.mult)
                        m2 = sb_m.tile([128, n], BF, tag="m2")
                        nc.gpsimd.tensor_tensor(
                            out=m2, in0=u2, in1=cq_prev[q], op=ALU.add)
                        nc.vector.scalar_tensor_tensor(
                            out=cq_new, in0=m2, scalar=0.5, in1=m1,
                            op0=ALU.mult, op1=ALU.add)
                    cq_prev[q] = cq_new

                    tcq = sb_tg.tile([128, n], BF, tag="tcq")
                    nc.scalar.activation(out=tcq, in_=cq_new, func=AF.Tanh,
                                         scale=0.5)
                    u3 = sb_m.tile([128, n], BF, tag="u3")
                    nc.gpsimd.tensor_tensor(out=u3, in0=tg[2], in1=tcq,
                                            op=ALU.mult)
                    hq = sb_hq.tile([128, n], BF, tag="hq")
                    nc.gpsimd.tensor_tensor(out=hq, in0=u3, in1=tcq,
                                            op=ALU.add)
                    hq_all[(q, s)] = hq

            # ---------------- phase B ----------------
            sums = []
            lgss = []
            for q in range(gq):
                lgq = ps_f.tile([128, n], F32, tag="f")
                for s in range(1, FT + 1):
                    for j in range(QUAD):
                        nc.tensor.matmul(
                            lgq[32 * j:32 * j + 32, :],
                            wp[32 * j:32 * j + 32, LGW + 32 * (s - 1):LGW + 32 * s],
                            hq_all[(q, s)][32 * j:32 * j + 32, :],
                            start=(s == 1), stop=(s == FT),
                            tile_position=(32 * j, 32 * j), skip_group_check=True)
                eoq = sb_e.tile([128, n], BF, tag="eoq")
                nc.scalar.activation(out=eoq, in_=lgq, func=AF.Exp,
                                     bias=fp[:, BOUTC:BOUTC + 1])
                lgs = sb_ph.tile([128, n], F32, tag="lgs")
                nc.scalar.activation(out=lgs, in_=lgq, func=AF.Identity,
                                     bias=fp[:, BOUTC:BOUTC + 1])
                lgss.append(lgs)
                soq = ps_f.tile([128, n], F32, tag="f")
                for j in range(QUAD):
                    nc.tensor.matmul(
                        soq[32 * j:32 * j + 32, :],
                        wp[32 * j:32 * j + 32, BO10:BO10 + 32],
                        eoq[32 * j:32 * j + 32, :],
                        start=True, stop=True,
                        tile_position=(32 * j, 32 * j), skip_group_check=True)
                sm = sb_ph.tile([128, n], F32, tag="sm")
                nc.vector.tensor_copy(sm, soq)
                sums.append(sm)

            for q in range(gq):
                ls = sb_ph.tile([128, n], F32, tag="ls")
                nc.scalar.activation(out=ls, in_=sums[q], func=AF.Ln)
                res = sb_ph.tile([128, n], F32, tag="res")
                nc.gpsimd.tensor_sub(out=res, in0=lgss[q], in1=ls)
                otp = ps_f.tile([128, 4 * 128], F32, tag="f")
                for bb in range(4):
                    nc.tensor.transpose(
                        otp[:, 128 * bb:128 * bb + 128],
                        res[:, 128 * bb:128 * bb + 128],
                        fp[:, IDT128:IDT128 + 128])
                oc = sb_oc.tile([128, 4 * 128], F32, tag="oc")
                nc.vector.tensor_copy(
                    oc.rearrange("p (j b so) -> p b j so", j=4, b=4),
                    otp.rearrange("p (b j so) -> p b j so", b=4, j=4))
                oc4 = oc.rearrange("p (j b so) -> p j b so", j=4, b=4)
                base = (g0 + q * QUAD) * n
                ov = out_d[base:base + QUAD * n, :, :].rearrange(
                    "(j b p) s o -> p j b (s o)", p=128, b=4)
                nc.sync.dma_start(out=ov, in_=oc4[:, :, :, 0:FT * OD])
            g0 += gsz


_PROGRAM_CACHE: dict[int, bass.Bass] = {}
_LAST_EXEC_NS = None
_LAST_RESULTS = None


def _get_program(Bshard: int) -> bass.Bass:
    if Bshard not in _PROGRAM_CACHE:
        _PROGRAM_CACHE[Bshard] = build_program(Bshard)
    return _PROGRAM_CACHE[Bshard]


def kernel(**inputs) -> np.ndarray:
    import ml_dtypes
    h_enc = np.asarray(inputs["h_enc"], np.float32)
    B = h_enc.shape[0]
    Bshard = B // N_CORES
    wp, fpk = _pack_weights(
        inputs["Wq"], inputs["bq"], inputs["Wk"], inputs["bk"],
        inputs["Wv"], inputs["bv"], inputs["W_ih"], inputs["b_ih"],
        inputs["W_hh"], inputs["b_hh"], inputs["W_out"], inputs["b_out"])
    wp_bf = wp.astype(ml_dtypes.bfloat16)
    nc = _get_program(Bshard)
    in_maps = []
    for c in range(N_CORES):
        in_maps.append({
            "h_enc": np.ascontiguousarray(h_enc[c * Bshard:(c + 1) * Bshard]),
            "wpack": wp_bf,
            "fpack": fpk,
        })
    import os
    trace = bool(os.environ.get("BASS_TRACE"))
    res = run_bass_kernel_spmd(nc, in_maps, list(range(N_CORES)), trace=trace)
    global _LAST_EXEC_NS, _LAST_RESULTS
    _LAST_EXEC_NS = res.exec_time_ns
    _LAST_RESULTS = res
    outs = [np.asarray(res.results[c]["out"]).reshape(Bshard, FT, OD)
            for c in range(N_CORES)]
    return np.concatenate(outs, axis=0).astype(np.float32)


# revision 7
# speedup vs baseline: 3.2827x; 1.0628x over previous
"""Trainium2 Bass kernel v2 for nn_Decoder: attention+LSTM decoder.

Math (reference):
  k = h_enc @ Wk.T + bk ; v = h_enc @ Wv.T + bv        [B, 8, 32]
  3 decoder steps: q = h @ Wq.T + bq
     score_t = q.k_t/sqrt(32) ; att = softmax_t
     ctx = sum_t att_t v_t ; (h, c) = LSTMCell(ctx, h, c)
  logits_s = h_s @ Wout.T + b_out ; out = log_softmax(logits)   [B, 3, 10]

v2 algebra (host-side folds):
  score_t = rho . x_t  with  rho = 0.5*A.T Hhat + w,
     A = (Wq.T Wk)/sqrt(H), w = (Wk.T bq)/sqrt(H)   (t-indep terms dropped)
  ctx = Wv xbar + bv with xbar = sum_t att_t x_t  -> k and v never computed.
  gates = M1 xbar + 0.5*W_hh Hhat + bg,  M1 = W_ih@Wv, bg = b_ih+b_hh+W_ih@bv
  sigmoid via tanh; factor-2 carries Chat=2c, Hhat=2h.

Layout: feature-major t-packed tiles [128, n]: partition = 32*(t%4)+h,
halves lo (t0-3) / hi (t4-7) at free offsets 0/n in one [128, 2n] tile.
Scores kept COMPACT: per quad of 4 chunks, scq[32*j + t, :] = score_t of
chunk j (rows t<8 of band j); one exp per quad-step instead of 8.
LSTM state packed per quad: cq/hq [128, n] band j = chunk j's Chat/Hhat.
Gates per role (i,f,o,g) packed per quad -> 1 tanh per role per quad-step.
"""

import numpy as np

import concourse.bass as bass
import concourse.bacc as bacc
import concourse.tile as tile
from concourse import mybir
from concourse.bass_utils import run_bass_kernel_spmd

H = 32
HT = 8
FT = 3
OD = 10
N_CORES = 8

BF = mybir.dt.bfloat16
F32 = mybir.dt.float32
AF = mybir.ActivationFunctionType
ALU = mybir.AluOpType

CHUNK = 512          # batch elements per chunk
QUAD = 4             # chunks per packed state quad
GROUP = 16           # chunks per group (phase A/B batching, >=QUAD, %QUAD==0)

# wpack (bf16, [128, WCOLS]) column layout
ID128 = 0            # 128: identity(128) for input transposes
RHO4 = 128           # 128: 0.5*[A,A,A,A] rows replicated per band
CMPT = 256           # 2*32: compact ones-reduce, half h at CMPT+32h
CMPW = 320           # 2*32: compact w-reduce
EXPB = 384           # 2*128: e-broadcast, half h at EXPB+128h
SUMB = 640           # 32: ssum (rows<8 ones, broadcast out)
I32S = 672           # 32: eye(32) per band (t-reduce for ubar)
GX = 704             # 4*32: (M1 gate-c rows).T per band, c in {i,f,o,g}
GH = 832             # 4*32: (0.5*W_hh gate-c rows).T per band
LGW = 960            # 3*32: logits lhsT per step s (cols 10s+o)
BO10 = 1056          # 32: block-ones(10) for softmax sums
WCOLS = 1088

# fpack (f32, [128, FCOLS])
BIASC = 0            # 4 cols: gate ACT bias per role c
BOUTC = 4            # 1 col: b_out pattern (rows 32j+10s+o), -30 on pad rows
WVEC = 5             # 1 col: w replicated per band (pt STT scalar)
IDT128 = 6           # 128: f32 eye(128) (output transposes)
FCOLS = 134

GATE_SL = (slice(0, 32), slice(32, 64), slice(96, 128), slice(64, 96))  # i,f,o,g


def _pack_weights(Wq, bq, Wk, bk, Wv, bv, W_ih, b_ih, W_hh, b_hh, W_out, b_out):
    Wq, bq, Wk, bk, Wv, bv, W_ih, b_ih, W_hh, b_hh, W_out, b_out = [
        np.asarray(a, np.float32) for a in
        (Wq, bq, Wk, bk, Wv, bv, W_ih, b_ih, W_hh, b_hh, W_out, b_out)]
    s = 1.0 / np.sqrt(np.float32(H))
    A = (Wq.T @ Wk) * s                    # [32,32]
    w = (Wk.T @ bq) * s                    # [32]
    M1 = W_ih @ Wv                         # [128,32]
    bg = b_ih + b_hh + W_ih @ bv           # [128] in (i,f,g,o) order

    wp = np.zeros((128, WCOLS), np.float32)
    wp[:, ID128:ID128 + 128] = np.eye(128)
    for r in range(4):
        P = slice(32 * r, 32 * r + 32)
        for c in range(4):
            wp[P, RHO4 + 32 * c:RHO4 + 32 * c + 32] = 0.5 * A
        for hf in range(2):
            wp[P, CMPT + 32 * hf + (4 * hf + r)] = 1.0
            wp[P, CMPW + 32 * hf + (4 * hf + r)] = w
            for c in range(4):
                wp[32 * r + (4 * hf + c),
                   EXPB + 128 * hf + 32 * c:EXPB + 128 * hf + 32 * c + 32] = 1.0
        wp[32 * r:32 * r + 8, SUMB:SUMB + 32] = 1.0
        wp[P, I32S:I32S + 32] = np.eye(32)
        for c, gsl in enumerate(GATE_SL):
            wp[P, GX + 32 * c:GX + 32 * c + 32] = M1[gsl].T
            wp[P, GH + 32 * c:GH + 32 * c + 32] = (0.5 * W_hh[gsl]).T
        for st in range(FT):
            for o in range(OD):
                wp[P, LGW + 32 * st + OD * st + o] = 0.5 * W_out[o]
        bo = np.zeros((32, 32), np.float32)
        for kk in range(30):
            for oo in range(30):
                if kk // OD == oo // OD:
                    bo[kk, oo] = 1.0
        bo[30, 30] = 1.0
        bo[31, 31] = 1.0
        wp[P, BO10:BO10 + 32] = bo

    fp = np.zeros((128, FCOLS), np.float32)
    for r in range(4):
        P = slice(32 * r, 32 * r + 32)
        for c in range(3):
            fp[P, BIASC + c] = 0.5 * bg[GATE_SL[c]]
        fp[P, BIASC + 3] = bg[GATE_SL[3]]
        bout = np.full(32, -30.0, np.float32)
        for st in range(FT):
            bout[OD * st:OD * st + OD] = b_out
        fp[P, BOUTC] = bout
        fp[P, WVEC] = w
    fp[:, IDT128:IDT128 + 128] = np.eye(128)
    return wp, fp


def build_program(Bshard: int) -> bass.Bass:
    assert Bshard % (QUAD * CHUNK) == 0
    nchunks = Bshard // CHUNK
    nc = bacc.Bacc(trn_type="TRN2")
    x_d = nc.declare_dram_parameter("h_enc", [Bshard, HT, H], F32, isOutput=False)
    wp_d = nc.declare_dram_parameter("wpack", [128, WCOLS], BF, isOutput=False)
    fp_d = nc.declare_dram_parameter("fpack", [128, FCOLS], F32, isOutput=False)
    out_d = nc.declare_dram_parameter("out", [Bshard, FT, OD], F32, isOutput=True)
    with tile.TileContext(nc) as tc:
        _body(nc, tc, x_d, wp_d, fp_d, out_d, nchunks, CHUNK)
    _split_matmul_waits(nc)
    nc.compile()
    return nc


def _split_matmul_waits(nc):
    """Walrus instruction structs fit one sync wait; move extras onto
    same-engine no-ops (each carrying a single wait) inserted just before."""
    for b in nc.m.functions[0].blocks:
        new = []
        for ins in b.instructions:
            si = ins.sync_info
            if (si is not None and len(si.on_wait) > 1
                    and not isinstance(ins, (mybir.InstEventSemaphore,
                                             mybir.InstNoOp))):
                for w in si.on_wait[:-1]:
                    nop = mybir.InstNoOp(
                        name=nc.get_next_instruction_name(), ins=[], outs=[],
                        engine=ins.engine,
                        sync_info=mybir.SyncInfo(on_wait=[w], on_update=[]))
                    nc.register_instruction(nop)
                    new.append(nop)
                ins.sync_info = mybir.SyncInfo(
                    on_wait=[si.on_wait[-1]], on_update=list(si.on_update))
            new.append(ins)
        b.instructions[:] = new


def _body(nc, tc, x_d, wp_d, fp_d, out_d, nchunks, n):
    from contextlib import ExitStack
    ctx = ExitStack()
    with ctx:
        singles = ctx.enter_context(tc.tile_pool(name="singles", bufs=1))
        sb_xb = ctx.enter_context(tc.tile_pool(name="sb_xb", bufs=3))
        sb_xt = ctx.enter_context(tc.tile_pool(name="sb_xt", bufs=GROUP + 2))
        sb_e = ctx.enter_context(tc.tile_pool(name="sb_e", bufs=6))
        sb_at = ctx.enter_context(tc.tile_pool(name="sb_at", bufs=10))
        sb_pt = ctx.enter_context(tc.tile_pool(name="sb_pt", bufs=8 * (GROUP // QUAD) + 2))
        sb_m = ctx.enter_context(tc.tile_pool(name="sb_m", bufs=2))
        sb_rho = ctx.enter_context(tc.tile_pool(name="sb_rho", bufs=3))
        sb_rs = ctx.enter_context(tc.tile_pool(name="sb_rs", bufs=5))
        sb_xq = ctx.enter_context(tc.tile_pool(name="sb_xq", bufs=6))
        sb_tg = ctx.enter_context(tc.tile_pool(name="sb_tg", bufs=5))
        sb_cq = ctx.enter_context(tc.tile_pool(name="sb_cq", bufs=GROUP // QUAD + 2))
        sb_hq = ctx.enter_context(tc.tile_pool(name="sb_hq", bufs=3 * (GROUP // QUAD) + 2))
        sb_ph = ctx.enter_context(tc.tile_pool(name="sb_ph", bufs=GROUP // QUAD + 1))
        sb_ph2 = ctx.enter_context(tc.tile_pool(name="sb_ph2", bufs=2))
        sb_oc = ctx.enter_context(tc.tile_pool(name="sb_oc", bufs=2))
        ps_x = ctx.enter_context(tc.tile_pool(name="ps_x", bufs=2, space="PSUM"))
        ps_f = ctx.enter_context(tc.tile_pool(name="ps_f", bufs=6, space="PSUM"))

        wp = singles.tile([128, WCOLS], BF)
        nc.sync.dma_start(out=wp, in_=wp_d[:, :])
        fp = singles.tile([128, FCOLS], F32)
        nc.sync.dma_start(out=fp, in_=fp_d[:, :])

        ident = wp[:, ID128:ID128 + 128]
        nquads = GROUP // QUAD

        g0 = 0
        while g0 < nchunks:
            gsz = min(GROUP, nchunks - g0)
            gq = gsz // QUAD
            # ---------------- phase A: load + transpose ----------------
            xts = []
            for cj in range(gsz):
                ci = g0 + cj
                xb = sb_xb.tile([128, 4, 256], BF, tag="xb")
                xv = x_d[ci * n:(ci + 1) * n].rearrange(
                    "(i p) t h -> p i (t h)", p=128)
                nc.gpsimd.dma_start(out=xb, in_=xv)
                xp = ps_x.tile([128, 2 * n], BF, tag="xp")
                for hf in range(2):
                    for i in range(4):
                        nc.tensor.transpose(
                            xp[:, n * hf + 128 * i:n * hf + 128 * i + 128],
                            xb[:, i, 128 * hf:128 * hf + 128],
                            ident)
                xt = sb_xt.tile([128, 2 * n], BF, tag="xt")
                nc.vector.tensor_copy(
                    xt[:, :].bitcast(mybir.dt.int32),
                    xp[:, :].bitcast(mybir.dt.int32))
                xts.append(xt)

            # ---------------- recurrent steps (stage-major) ----------------
            hq_all = {}
            cq_prev = {}
            for s in range(1, FT + 1):
                # S1: rho MMs + rho copy (ACT, +w bias) + pt TTs
                pt = {}
                if s > 1:
                    for q in range(gq):
                        jj = [q * QUAD + j for j in range(QUAD)]
                        hq_prev = hq_all[(q, s - 1)]
                        rps = []
                        for j in range(QUAD):
                            rp = ps_f.tile([128, n], F32, tag="f")
                            nc.tensor.matmul(
                                rp[:, :], wp[32 * j:32 * j + 32, RHO4:RHO4 + 128],
                                hq_prev[32 * j:32 * j + 32, :],
                                start=True, stop=True,
                                tile_position=(32 * j, 0), skip_group_check=True)
                            rps.append(rp)
                        for j in range(QUAD):
                            rsb = sb_rho.tile([128, n], BF, tag="rsb")
                            nc.scalar.activation(
                                out=rsb, in_=rps[j], func=AF.Identity,
                                bias=fp[:, WVEC:WVEC + 1])
                            for hf in range(2):
                                t = sb_pt.tile([128, n], BF, tag="pt")
                                nc.vector.tensor_tensor(
                                    out=t, in0=rsb,
                                    in1=xts[jj[j]][:, n * hf:n * hf + n],
                                    op=ALU.mult)
                                pt[(q, j, hf)] = t

                # S2: score volleys
                scqs = []
                for q in range(gq):
                    jj = [q * QUAD + j for j in range(QUAD)]
                    scq = ps_f.tile([128, n], F32, tag="f")
                    for k in range(2):
                        for j in range(QUAD):
                            if s > 1:
                                lh = wp[0:128, CMPT + 32 * k:CMPT + 32 * k + 32]
                                rh = pt[(q, j, k)][:, :]
                            else:
                                lh = wp[0:128, CMPW + 32 * k:CMPW + 32 * k + 32]
                                rh = xts[jj[j]][:, n * k:n * k + n]
                            nc.tensor.matmul(
                                scq[32 * j:32 * j + 32, :], lh, rh,
                                start=(k == 0), stop=(k == 1),
                                tile_position=(0, 32 * j), skip_group_check=True)
                    scqs.append(scq)

                # S3: exp
                escs = []
                for q in range(gq):
                    esc = sb_e.tile([128, n], BF, tag="esc")
                    nc.scalar.activation(out=esc, in_=scqs[q], func=AF.Exp)
                    escs.append(esc)

                # S4: ssum + recip
                rss = []
                for q in range(gq):
                    ssq = ps_f.tile([128, n], F32, tag="f")
                    for j in range(QUAD):
                        nc.tensor.matmul(
                            ssq[32 * j:32 * j + 32, :],
                            wp[32 * j:32 * j + 32, SUMB:SUMB + 32],
                            escs[q][32 * j:32 * j + 32, :],
                            start=True, stop=True,
                            tile_position=(32 * j, 32 * j), skip_group_check=True)
                    rs = sb_rs.tile([128, n], F32, tag="rs")
                    nc.vector.reciprocal_approx_fast(out=rs, in_=ssq)
                    rss.append(rs)

                # S5: e-broadcast + at products + ubar reduce + xbar (per quad)
                xqs = []
                for q in range(gq):
                    jj = [q * QUAD + j for j in range(QUAD)]
                    at = {}
                    for hf in range(2):
                        for j in range(QUAD):
                            ebp = ps_f.tile([128, n], F32, tag="f")
                            nc.tensor.matmul(
                                ebp[:, :],
                                wp[32 * j:32 * j + 32,
                                   EXPB + 128 * hf:EXPB + 128 * hf + 128],
                                escs[q][32 * j:32 * j + 32, :],
                                start=True, stop=True,
                                tile_position=(32 * j, 0), skip_group_check=True)
                            t = sb_at.tile([128, n], BF, tag="at")
                            nc.vector.tensor_tensor(
                                out=t, in0=ebp,
                                in1=xts[jj[j]][:, n * hf:n * hf + n],
                                op=ALU.mult)
                            at[(j, hf)] = t
                    ubq = ps_f.tile([128, n], F32, tag="f")
                    for hf in range(2):
                        for j in range(QUAD):
                            nc.tensor.matmul(
                                ubq[32 * j:32 * j + 32, :],
                                wp[0:128, I32S:I32S + 32], at[(j, hf)][:, :],
                                start=(hf == 0), stop=(hf == 1),
                                tile_position=(0, 32 * j), skip_group_check=True)
                    xq = sb_xq.tile([128, n], BF, tag="xq")
                    nc.vector.tensor_tensor(out=xq, in0=ubq, in1=rss[q],
                                            op=ALU.mult)
                    xqs.append(xq)

                # S7: gates + tanh
                tgs = {}
                for q in range(gq):
                    hq_prev = hq_all.get((q, s - 1))
                    gps = []
                    for _gi in range(4):
                        gp_t = ps_f.tile([128, n], F32, tag="f")
                        gps.append(gp_t)
                    for c in range(4):
                        for j in range(QUAD):
                            nc.tensor.matmul(
                                gps[c][32 * j:32 * j + 32, :],
                                wp[32 * j:32 * j + 32, GX + 32 * c:GX + 32 * c + 32],
                                xqs[q][32 * j:32 * j + 32, :],
                                start=True, stop=(s == 1),
                                tile_position=(32 * j, 32 * j),
                                skip_group_check=True)
                    if s > 1:
                        for c in range(4):
                            for j in range(QUAD):
                                nc.tensor.matmul(
                                    gps[c][32 * j:32 * j + 32, :],
                                    wp[32 * j:32 * j + 32,
                                       GH + 32 * c:GH + 32 * c + 32],
                                    hq_prev[32 * j:32 * j + 32, :],
                                    start=False, stop=True,
                                    tile_position=(32 * j, 32 * j),
                                    skip_group_check=True)
                    for c in range(4):
                        t = sb_tg.tile([128, n], BF, tag=f"tg{c}")
                        if c < 3:
                            nc.scalar.activation(
                                out=t, in_=gps[c], func=AF.Tanh, scale=0.5,
                                bias=fp[:, BIASC + c:BIASC + c + 1])
                        else:
                            nc.scalar.activation(
                                out=t, in_=gps[c], func=AF.Tanh,
                                bias=fp[:, BIASC + 3:BIASC + 4])
                        tgs[(q, c)] = t

                # S8: LSTM elementwise
                for q in range(gq):
                    tg = [tgs[(q, c)] for c in range(4)]
                    cq_new = sb_cq.tile([128, n], BF, tag="cq")
                    if s == 1:
                        nc.vector.scalar_tensor_tensor(
                            out=cq_new, in0=tg[0], scalar=1.0, in1=tg[3],
                            op0=ALU.add, op1=ALU.mult)
                    else:
                        m1 = sb_m.tile([128, n], BF, tag="m1")
                        nc.vector.scalar_tensor_tensor(
                            out=m1, in0=tg[0], scalar=1.0, in1=tg[3],
                            op0=ALU.add, op1=ALU.mult)
                        u2 = sb_m.tile([128, n], BF, tag="u2")
                        nc.gpsimd.tensor_tensor(
                            out=u2, in0=tg[1], in1=cq_prev[q], op=ALU.mult)
                        m2 = sb_m.tile([128, n], BF, tag="m2")
                        nc.gpsimd.tensor_tensor(
                            out=m2, in0=u2, in1=cq_prev[q], op=ALU.add)
                        nc.vector.scalar_tensor_tensor(
                            out=cq_new, in0=m2, scalar=0.5, in1=m1,
                            op0=ALU.mult, op1=ALU.add)
                    cq_prev[q] = cq_new

                    tcq = sb_tg.tile([128, n], BF, tag="tcq")
                    nc.scalar.activation(out=tcq, in_=cq_new, func=AF.Tanh,
                                         scale=0.5)
                    u3 = sb_m.tile([128, n], BF, tag="u3")
                    nc.gpsimd.tensor_tensor(out=u3, in0=tg[2], in1=tcq,
                                            op=ALU.mult)
                    hq = sb_hq.tile([128, n], BF, tag="hq")
                    nc.gpsimd.tensor_tensor(out=hq, in0=u3, in1=tcq,
                                            op=ALU.add)
                    hq_all[(q, s)] = hq

            # ---------------- phase B ----------------
            sums = []
            lgss = []
            for q in range(gq):
                lgq = ps_f.tile([128, n], F32, tag="f")
                for s in range(1, FT + 1):
                    for j in range(QUAD):
                        nc.tensor.matmul(
                            lgq[32 * j:32 * j + 32, :],
                            wp[32 * j:32 * j + 32, LGW + 32 * (s - 1):LGW + 32 * s],
                            hq_all[(q, s)][32 * j:32 * j + 32, :],
                            start=(s == 1), stop=(s == FT),
                            tile_position=(32 * j, 32 * j), skip_group_check=True)
                eoq = sb_e.tile([128, n], BF, tag="eoq")
                nc.scalar.activation(out=eoq, in_=lgq, func=AF.Exp,
                                     bias=fp[:, BOUTC:BOUTC + 1])
                lgs = sb_ph.tile([128, n], F32, tag="lgs")
                nc.scalar.activation(out=lgs, in_=lgq, func=AF.Identity,
                                     bias=fp[:, BOUTC:BOUTC + 1])
                lgss.append(lgs)
                soq = ps_f.tile([128, n], F32, tag="f")
                for j in range(QUAD):
                    nc.tensor.matmul(
                        soq[32 * j:32 * j + 32, :],
                        wp[32 * j:32 * j + 32, BO10:BO10 + 32],
                        eoq[32 * j:32 * j + 32, :],
                        start=True, stop=True,
                        tile_position=(32 * j, 32 * j), skip_group_check=True)
                sm = sb_ph.tile([128, n], F32, tag="sm")
                nc.vector.tensor_copy(sm, soq)
                sums.append(sm)

            for q in range(gq):
                ls = sb_ph2.tile([128, n], F32, tag="ls")
                nc.scalar.activation(out=ls, in_=sums[q], func=AF.Ln)
                res = sb_ph2.tile([128, n], F32, tag="res")
                nc.gpsimd.tensor_sub(out=res, in0=lgss[q], in1=ls)
                otp = ps_f.tile([128, 4 * 128], F32, tag="f")
                for bb in range(4):
                    nc.tensor.transpose(
                        otp[:, 128 * bb:128 * bb + 128],
                        res[:, 128 * bb:128 * bb + 128],
                        fp[:, IDT128:IDT128 + 128])
                oc = sb_oc.tile([128, 4 * 128], F32, tag="oc")
                nc.vector.tensor_copy(
                    oc.rearrange("p (j b so) -> p b j so", j=4, b=4),
                    otp.rearrange("p (b j so) -> p b j so", b=4, j=4))
                oc4 = oc.rearrange("p (j b so) -> p j b so", j=4, b=4)
                base = (g0 + q * QUAD) * n
                ov = out_d[base:base + QUAD * n, :, :].rearrange(
                    "(j b p) s o -> p j b (s o)", p=128, b=4)
                nc.sync.dma_start(out=ov, in_=oc4[:, :, :, 0:FT * OD])
            g0 += gsz


_PROGRAM_CACHE: dict[int, bass.Bass] = {}
_LAST_EXEC_NS = None
_LAST_RESULTS = None


def _get_program(Bshard: int) -> bass.Bass:
    if Bshard not in _PROGRAM_CACHE:
        _PROGRAM_CACHE[Bshard] = build_program(Bshard)
    return _PROGRAM_CACHE[Bshard]


def kernel(**inputs) -> np.ndarray:
    import ml_dtypes
    h_enc = np.asarray(inputs["h_enc"], np.float32)
    B = h_enc.shape[0]
    Bshard = B // N_CORES
    wp, fpk = _pack_weights(
        inputs["Wq"], inputs["bq"], inputs["Wk"], inputs["bk"],
        inputs["Wv"], inputs["bv"], inputs["W_ih"], inputs["b_ih"],
        inputs["W_hh"], inputs["b_hh"], inputs["W_out"], inputs["b_out"])
    wp_bf = wp.astype(ml_dtypes.bfloat16)
    nc = _get_program(Bshard)
    in_maps = []
    for c in range(N_CORES):
        in_maps.append({
            "h_enc": np.ascontiguousarray(h_enc[c * Bshard:(c + 1) * Bshard]),
            "wpack": wp_bf,
            "fpack": fpk,
        })
    import os
    trace = bool(os.environ.get("BASS_TRACE"))
    res = run_bass_kernel_spmd(nc, in_maps, list(range(N_CORES)), trace=trace)
    global _LAST_EXEC_NS, _LAST_RESULTS
    _LAST_EXEC_NS = res.exec_time_ns
    _LAST_RESULTS = res
    outs = [np.asarray(res.results[c]["out"]).reshape(Bshard, FT, OD)
            for c in range(N_CORES)]
    return np.concatenate(outs, axis=0).astype(np.float32)
